# revision 3
# baseline (speedup 1.0000x reference)
"""MatchingNetwork forward on 8 Trainium2 NeuronCores.

The reference network's output reduces exactly to one_hot(labels, V) in f32:
the final einsum('btn,btv->btv', att, one_hot) sums att over n, and att is a
softmax over n, so the output is one_hot scaled by sum(softmax) == 1 (to float
rounding, ~1e-7).  Everything upstream (embedding gathers, BiLSTM GLayer,
attentional FLayer) cancels out of the result for every input.

So the kernel is a distributed one-hot materialization: B*T = 2048 rows of
V = 32000 each, data-parallel over rows across 8 cores (256 rows/core).
All output values are exactly 0 or 1, so the device writes uint8 (8.19
MB/core instead of 32.77 MB f32 -- the whole job is HBM-write-bound and the
8 cores together saturate the chip's HBM) and the host casts back to f32
losslessly.

Per core: labels arrive as [128 partitions, 2] f32 (row r = b*128 + p).
Iota rows arrive as inputs (a 1000-wide ramp tile and a 4000-wide main tile)
so no gpsimd iota generation sits on the critical path.  Compares are
DVE/gpsimd tensor_scalar(subtract, is_equal) -> uint8 tiles that stream to
DRAM on the two HWDGE queues (sync + scalar).  DVE ops have a ~0.9-2 us
fixed+width cost, so 4 of the 20 compare tiles are offloaded to gpsimd to
keep compute ahead of the DMA stream.
"""

import os
import sys

for _p in ("/opt/trn_rl_repo", "/root/.axon_site/_ro/trn_rl_repo"):
    if os.path.isdir(_p) and _p not in sys.path:
        sys.path.append(_p)

import numpy as np

B, T, V = 32, 64, 32000
N_CORES = 8
ROWS = B * T                 # 2048 one-hot rows total
RPC = ROWS // N_CORES        # 256 rows per core
NB = RPC // 128              # 2 batches of 128 partitions

RAMP_W = 1000                # ramp tile width (also the ramp iota width)
WIDE_W = 4000                # steady-state tile width (512 KB uint8 DMAs)
N_RAMP = 2                   # ramp tiles per batch: cols [0, 2000)
N_WIDE = 7                   # wide tiles per batch: cols [2000, 30000)
TAIL_W = 2000                # tail tile: cols [30000, 32000)
GPSIMD_WIDES = (4, 5)        # wide-tile indices computed on gpsimd, not DVE

_cache = {}


def _build_nc():
    import concourse.bacc as bacc
    import concourse.mybir as mybir
    from concourse.tile import TileContext

    f32 = mybir.dt.float32
    u8 = mybir.dt.uint8
    nc = bacc.Bacc()
    lab_d = nc.dram_tensor("labels", [128, NB], f32, kind="ExternalInput")
    ramp_d = nc.dram_tensor("iota_ramp", [128, RAMP_W], f32,
                            kind="ExternalInput")
    full_d = nc.dram_tensor("iota_full", [128, WIDE_W], f32,
                            kind="ExternalInput")
    out_d = nc.dram_tensor("out", [NB, 128, V], u8, kind="ExternalOutput")

    with TileContext(nc) as tc:
        with tc.tile_pool(name="const", bufs=1) as cpool, \
             tc.tile_pool(name="work", bufs=8) as wpool, \
             tc.tile_pool(name="gwork", bufs=3) as gpool:
            lab = cpool.tile([128, NB], f32, tag="lab")
            ramp = cpool.tile([128, RAMP_W], f32, tag="ramp")
            full = cpool.tile([128, WIDE_W], f32, tag="full")
            # labels + ramp iota first (small, land ~2 us after issue);
            # the 2 MB full iota follows on the other queue.
            nc.sync.dma_start(out=lab[:, :], in_=lab_d[:, :])
            nc.scalar.dma_start(out=ramp[:, :], in_=ramp_d[:, :])
            nc.sync.dma_start(out=full[:, :], in_=full_d[:, :])
            dma_engines = [nc.sync, nc.scalar]

            def emit(col, w, iota, pool, ceng):
                # iota holds values (global_col - col) for cols [col, col+w)
                for b in range(NB):
                    o = pool.tile([128, WIDE_W], u8, tag="o")
                    # o = is_equal(iota - (-col), lab[:, b])
                    ceng.tensor_scalar(
                        out=o[:, :w], in0=iota,
                        scalar1=float(-col), scalar2=lab[:, b:b + 1],
                        op0=mybir.AluOpType.subtract,
                        op1=mybir.AluOpType.is_equal)
                    dma_engines[b].dma_start(
                        out=out_d[b, :, col:col + w], in_=o[:, :w])

            for r in range(N_RAMP):
                emit(r * RAMP_W, RAMP_W, ramp[:, :RAMP_W], wpool, nc.vector)
            for k in range(N_WIDE):
                c = N_RAMP * RAMP_W + k * WIDE_W
                if k in GPSIMD_WIDES:
                    emit(c, WIDE_W, full[:, :WIDE_W], gpool, nc.gpsimd)
                else:
                    emit(c, WIDE_W, full[:, :WIDE_W], wpool, nc.vector)
            emit(N_RAMP * RAMP_W + N_WIDE * WIDE_W, TAIL_W,
                 full[:, :TAIL_W], wpool, nc.vector)
    nc.finalize()
    return nc


def kernel(**inputs):
    from concourse.bass_utils import run_bass_kernel_spmd

    if "nc" not in _cache:
        _cache["nc"] = _build_nc()
    nc = _cache["nc"]

    # Label values < 2^24 are exact in f32.
    lab = np.asarray(inputs["labels"]).reshape(-1).astype(np.float32)
    ramp = np.ascontiguousarray(
        np.broadcast_to(np.arange(RAMP_W, dtype=np.float32), (128, RAMP_W)))
    full = np.ascontiguousarray(
        np.broadcast_to(np.arange(WIDE_W, dtype=np.float32), (128, WIDE_W)))
    in_maps = []
    for i in range(N_CORES):
        shard = lab[i * RPC:(i + 1) * RPC].reshape(NB, 128).T  # [128, NB]
        in_maps.append({"labels": np.ascontiguousarray(shard),
                        "iota_ramp": ramp, "iota_full": full})

    trace = bool(int(os.environ.get("BASS_KERNEL_TRACE", "0")))
    res = run_bass_kernel_spmd(nc, in_maps, list(range(N_CORES)), trace=trace)
    _cache["last_res"] = res

    outs = [res.results[i]["out"].reshape(RPC, V) for i in range(N_CORES)]
    return np.concatenate(outs, axis=0).reshape(B, T, V).astype(np.float32)


# revision 5
# speedup vs baseline: 5.5346x; 5.5346x over previous
"""MatchingNetwork forward on 8 Trainium2 NeuronCores.

The reference network's output reduces exactly to one_hot(labels, V) in f32:
the final einsum('btn,btv->btv', att, one_hot) sums att over n, and att is a
softmax over n, so the output is one_hot scaled by sum(softmax) == 1 (to float
rounding, ~1e-7).  Everything upstream (embedding gathers, BiLSTM GLayer,
attentional FLayer) cancels out of the result for every input.

So the kernel is a distributed one-hot materialization: B*T = 2048 rows of
V = 32000 each, data-parallel over rows across 8 cores (256 rows/core).
All output values are exactly 0 or 1, so the device writes uint8 (8.19
MB/core instead of 32.77 MB f32 -- the whole job is HBM-write-bound and the
8 cores together saturate the chip's HBM) and the host casts back to f32
losslessly.

Per core: labels arrive as [128 partitions, 2] f32 (row r = b*128 + p).
Iota rows arrive as uint16 inputs (a 1000-wide ramp tile and a 4000-wide
main tile) so nothing but the tiny label load gates the first write.  All
compares run on DVE: tensor_scalar(out=u8, add(iota, col), is_equal, lab)
-- uint16 in0 keeps DVE in its fastest perf mode and halves iota load
traffic; all values stay < 2^16 so the unsigned math is exact.  uint8
1/0 tiles stream to DRAM on the two HWDGE queues (sync + scalar).
gpsimd is kept off the data path entirely (its tensor_scalar ucode is
~60x slower and stalls concurrent DVE ops).
"""

import os
import sys

for _p in ("/opt/trn_rl_repo", "/root/.axon_site/_ro/trn_rl_repo"):
    if os.path.isdir(_p) and _p not in sys.path:
        sys.path.append(_p)

import numpy as np

B, T, V = 32, 64, 32000
N_CORES = 8
ROWS = B * T                 # 2048 one-hot rows total
RPC = ROWS // N_CORES        # 256 rows per core
NB = RPC // 128              # 2 batches of 128 partitions

RAMP_W = 1000                # ramp tile width (also the ramp iota width)
WIDE_W = 4000                # steady-state tile width (512 KB uint8 DMAs)
N_RAMP = 2                   # ramp tiles per batch: cols [0, 2000)
N_WIDE = 7                   # wide tiles per batch: cols [2000, 30000)
TAIL_W = 2000                # tail tile: cols [30000, 32000)

_cache = {}


def _build_nc():
    import concourse.bacc as bacc
    import concourse.mybir as mybir
    from concourse.tile import TileContext

    f32 = mybir.dt.float32
    u16 = mybir.dt.uint16
    u8 = mybir.dt.uint8
    nc = bacc.Bacc()
    lab_d = nc.dram_tensor("labels", [128, NB], f32, kind="ExternalInput")
    ramp_d = nc.dram_tensor("iota_ramp", [128, RAMP_W], u16,
                            kind="ExternalInput")
    full_d = nc.dram_tensor("iota_full", [128, WIDE_W], u16,
                            kind="ExternalInput")
    out_d = nc.dram_tensor("out", [NB, 128, V], u8, kind="ExternalOutput")

    with TileContext(nc) as tc:
        with tc.tile_pool(name="const", bufs=1) as cpool, \
             tc.tile_pool(name="work", bufs=8) as wpool:
            lab = cpool.tile([128, NB], f32, tag="lab")
            ramp = cpool.tile([128, RAMP_W], u16, tag="ramp")
            full = cpool.tile([128, WIDE_W], u16, tag="full")
            # labels + ramp iota first (small, land ~2 us after issue);
            # the 1 MB full iota follows on the sync queue.
            nc.sync.dma_start(out=lab[:, :], in_=lab_d[:, :])
            nc.scalar.dma_start(out=ramp[:, :], in_=ramp_d[:, :])
            nc.sync.dma_start(out=full[:, :], in_=full_d[:, :])
            dma_engines = [nc.sync, nc.scalar]

            def emit(col, w, iota):
                # iota holds values (global_col - col) for cols [col, col+w)
                for b in range(NB):
                    o = wpool.tile([128, WIDE_W], u8, tag="o")
                    # o = is_equal(iota + col, lab[:, b]); all < 2^16, exact
                    nc.vector.tensor_scalar(
                        out=o[:, :w], in0=iota,
                        scalar1=float(col), scalar2=lab[:, b:b + 1],
                        op0=mybir.AluOpType.add,
                        op1=mybir.AluOpType.is_equal)
                    dma_engines[b].dma_start(
                        out=out_d[b, :, col:col + w], in_=o[:, :w])

            for r in range(N_RAMP):
                emit(r * RAMP_W, RAMP_W, ramp[:, :RAMP_W])
            for k in range(N_WIDE):
                emit(N_RAMP * RAMP_W + k * WIDE_W, WIDE_W, full[:, :WIDE_W])
            emit(N_RAMP * RAMP_W + N_WIDE * WIDE_W, TAIL_W, full[:, :TAIL_W])
    nc.finalize()
    return nc


def kernel(**inputs):
    from concourse.bass_utils import run_bass_kernel_spmd

    if "nc" not in _cache:
        _cache["nc"] = _build_nc()
    nc = _cache["nc"]

    # Label values < 2^24 are exact in f32 (scalar2 APs must be f32).
    lab = np.asarray(inputs["labels"]).reshape(-1).astype(np.float32)
    ramp = np.ascontiguousarray(
        np.broadcast_to(np.arange(RAMP_W, dtype=np.uint16), (128, RAMP_W)))
    full = np.ascontiguousarray(
        np.broadcast_to(np.arange(WIDE_W, dtype=np.uint16), (128, WIDE_W)))
    in_maps = []
    for i in range(N_CORES):
        shard = lab[i * RPC:(i + 1) * RPC].reshape(NB, 128).T  # [128, NB]
        in_maps.append({"labels": np.ascontiguousarray(shard),
                        "iota_ramp": ramp, "iota_full": full})

    trace = bool(int(os.environ.get("BASS_KERNEL_TRACE", "0")))
    res = run_bass_kernel_spmd(nc, in_maps, list(range(N_CORES)), trace=trace)
    _cache["last_res"] = res

    outs = [res.results[i]["out"].reshape(RPC, V) for i in range(N_CORES)]
    return np.concatenate(outs, axis=0).reshape(B, T, V).astype(np.float32)


# revision 8
# speedup vs baseline: 5.9845x; 1.0813x over previous
"""MatchingNetwork forward on 8 Trainium2 NeuronCores.

The reference network's output reduces exactly to one_hot(labels, V) in f32:
the final einsum('btn,btv->btv', att, one_hot) sums att over n, and att is a
softmax over n, so the output is one_hot scaled by sum(softmax) == 1 (to float
rounding, ~1e-7).  Everything upstream (embedding gathers, BiLSTM GLayer,
attentional FLayer) cancels out of the result for every input.

So the kernel is a distributed one-hot materialization: B*T = 2048 rows of
V = 32000 each, data-parallel over rows across 8 cores (256 rows/core).
All output values are exactly 0 or 1, so the device writes uint8 (8.19
MB/core instead of 32.77 MB f32 -- the whole job is HBM-write-bound and the
8 cores together saturate the chip's HBM) and the host casts back to f32
losslessly.

Strategy: the output is zeros except ONE byte per row.  The kernel streams
zeros for the whole output from a single memset SBUF tile -- these DMAs have
no data dependencies, so both HWDGE queues saturate right after the preamble
with no iota/label/compute on the critical path (a DVE compare pipeline
caps at ~0.52 ns/col for uint8 out = ~35 us; pure DMA streaming avoids it).
The 256 ones then land via 2 indirect scatter DMAs: the host (which sees the
labels) pre-builds one 500-byte block per row holding that row's one-hot
segment, plus the block index (row*64 + label//500; 500 divides V so blocks
never straddle rows).  Tile's dependency tracking orders the scatters after
the zero stream (WAW on the output tensor).
"""

import os
import sys

for _p in ("/opt/trn_rl_repo", "/root/.axon_site/_ro/trn_rl_repo"):
    if os.path.isdir(_p) and _p not in sys.path:
        sys.path.append(_p)

import numpy as np

B, T, V = 32, 64, 32000
N_CORES = 8
ROWS = B * T                 # 2048 one-hot rows total
RPC = ROWS // N_CORES        # 256 rows per core
NB = RPC // 128              # 2 batches of 128 partitions

BLK = 500                    # patch block size; BLK | V so blocks stay in-row
NBLK = V // BLK              # 64 blocks per row
CHUNK = 4000                 # zero-stream tile width (512 KB uint8 DMAs)
NCH = V // CHUNK             # 8 chunks per batch
CB = CHUNK // BLK            # 8 block-rows per chunk

_cache = {}


def _build_nc():
    import concourse.bacc as bacc
    import concourse.mybir as mybir
    from concourse import bass
    from concourse.tile import TileContext

    i32 = mybir.dt.int32
    u8 = mybir.dt.uint8
    nc = bacc.Bacc()
    patch_d = [nc.dram_tensor(f"patch{b}", [128, BLK], u8,
                              kind="ExternalInput") for b in range(NB)]
    idx_d = [nc.dram_tensor(f"idx{b}", [128, 1], i32,
                            kind="ExternalInput") for b in range(NB)]
    out_d = nc.dram_tensor("out", [NB * 128, NBLK, BLK], u8,
                           kind="ExternalOutput")

    with TileContext(nc) as tc:
        with tc.tile_pool(name="const", bufs=1) as cpool:
            zt = cpool.tile([128, CHUNK], u8, tag="zt")
            nc.vector.memset(zt[:, :], 0)
            patch = [cpool.tile([128, BLK], u8, name=f"patch_t{b}")
                     for b in range(NB)]
            idx = [cpool.tile([128, 1], i32, name=f"idx_t{b}")
                   for b in range(NB)]
            for b in range(NB):
                nc.scalar.dma_start(out=patch[b][:, :], in_=patch_d[b][:, :])
                nc.scalar.dma_start(out=idx[b][:, :], in_=idx_d[b][:, :])
            dma_engines = [nc.sync, nc.scalar]
            # Zero stream: 16 independent 512 KB DMAs from the memset tile.
            for c in range(NCH):
                for b in range(NB):
                    dma_engines[b].dma_start(
                        out=out_d[b * 128:(b + 1) * 128,
                                  c * CB:(c + 1) * CB, :],
                        in_=zt[:, :])
            # Patch the 256 ones: per batch, partition p writes its 500-byte
            # block at block index idx[p] (= global row * 64 + label // 500).
            for b in range(NB):
                nc.gpsimd.indirect_dma_start(
                    out=out_d[:, :, :],
                    out_offset=bass.IndirectOffsetOnAxis(
                        ap=idx[b][:, :1], axis=1),
                    in_=patch[b][:, :],
                    in_offset=None)
    nc.finalize()
    return nc


def kernel(**inputs):
    from concourse.bass_utils import run_bass_kernel_spmd

    if "nc" not in _cache:
        _cache["nc"] = _build_nc()
    nc = _cache["nc"]

    lab = np.asarray(inputs["labels"]).reshape(-1).astype(np.int64)
    in_maps = []
    for i in range(N_CORES):
        shard = lab[i * RPC:(i + 1) * RPC].reshape(NB, 128)  # [NB, 128]
        im = {}
        for b in range(NB):
            lb = shard[b]
            patch = np.zeros((128, BLK), dtype=np.uint8)
            patch[np.arange(128), lb % BLK] = 1
            gidx = ((b * 128 + np.arange(128)) * NBLK + lb // BLK)
            im[f"patch{b}"] = patch
            im[f"idx{b}"] = gidx.astype(np.int32).reshape(128, 1)
        in_maps.append(im)

    trace = bool(int(os.environ.get("BASS_KERNEL_TRACE", "0")))
    res = run_bass_kernel_spmd(nc, in_maps, list(range(N_CORES)), trace=trace)
    _cache["last_res"] = res

    outs = [res.results[i]["out"].reshape(RPC, V) for i in range(N_CORES)]
    return np.concatenate(outs, axis=0).reshape(B, T, V).astype(np.float32)


# revision 10
# speedup vs baseline: 7.0624x; 1.1801x over previous
"""MatchingNetwork forward on 8 Trainium2 NeuronCores.

The reference network's output reduces exactly to one_hot(labels, V) in f32:
the final einsum('btn,btv->btv', att, one_hot) sums att over n, and att is a
softmax over n, so the output is one_hot scaled by sum(softmax) == 1 (to float
rounding, ~1e-7).  Everything upstream (embedding gathers, BiLSTM GLayer,
attentional FLayer) cancels out of the result for every input.

So the kernel is a distributed one-hot materialization: B*T = 2048 rows of
V = 32000 each, data-parallel over rows across 8 cores (256 rows/core).
All output values are exactly 0 or 1, so the device writes uint8 (8.19
MB/core instead of 32.77 MB f32 -- the whole job is HBM-write-bound and the
8 cores together saturate the chip's HBM) and the host casts back to f32
losslessly.

Strategy: the output is zeros except ONE byte per row.  The kernel streams
zeros for the whole output from a single memset SBUF tile -- these DMAs have
no data dependencies, so both HWDGE queues saturate right after the preamble
with no iota/label/compute on the critical path (a DVE compare pipeline
caps at ~0.52 ns/col for uint8 out = ~35 us; pure DMA streaming avoids it).
The 256 ones then land via 2 indirect scatter DMAs: the host (which sees the
labels) pre-builds one 500-byte block per row holding that row's one-hot
segment, plus the block index (row*64 + label//500; 500 divides V so blocks
never straddle rows).  Tile's dependency tracking orders the scatters after
the zero stream (WAW on the output tensor).
"""

import os
import sys

for _p in ("/opt/trn_rl_repo", "/root/.axon_site/_ro/trn_rl_repo"):
    if os.path.isdir(_p) and _p not in sys.path:
        sys.path.append(_p)

import numpy as np

B, T, V = 32, 64, 32000
N_CORES = 8
ROWS = B * T                 # 2048 one-hot rows total
RPC = ROWS // N_CORES        # 256 rows per core
NB = RPC // 128              # 2 batches of 128 partitions

BLK = 500                    # patch block size; BLK | V so blocks stay in-row
NBLK = V // BLK              # 64 blocks per row
CHUNK = 4000                 # zero-stream tile width (512 KB uint8 DMAs)
NCH = V // CHUNK             # 8 chunks per batch
CB = CHUNK // BLK            # 8 block-rows per chunk

_cache = {}


def _build_nc():
    import concourse.bacc as bacc
    import concourse.mybir as mybir
    from concourse import bass
    from concourse.tile import TileContext

    i32 = mybir.dt.int32
    u32 = mybir.dt.uint32
    u8 = mybir.dt.uint8
    nc = bacc.Bacc()
    patch_d = [nc.dram_tensor(f"patch{b}", [128, BLK], u8,
                              kind="ExternalInput") for b in range(NB)]
    idx_d = [nc.dram_tensor(f"idx{b}", [128, 1], i32,
                            kind="ExternalInput") for b in range(NB)]
    out_d = nc.dram_tensor("out", [NB * 128, NBLK, BLK], u8,
                           kind="ExternalOutput")

    with TileContext(nc) as tc:
        with tc.tile_pool(name="const", bufs=1) as cpool:
            # u32 view quadruples DVE memset throughput (2-byte/4-byte
            # dtypes run packed perf modes; u8 memset runs 1x).
            zt = cpool.tile([128, CHUNK // 4], u32, tag="zt")
            nc.vector.memset(zt[:, :], 0)
            patch = [cpool.tile([128, BLK], u8, name=f"patch_t{b}")
                     for b in range(NB)]
            idx = [cpool.tile([128, 1], i32, name=f"idx_t{b}")
                   for b in range(NB)]
            for b in range(NB):
                nc.scalar.dma_start(out=patch[b][:, :], in_=patch_d[b][:, :])
                nc.scalar.dma_start(out=idx[b][:, :], in_=idx_d[b][:, :])
            dma_engines = [nc.sync, nc.scalar]
            # Zero stream: 16 independent 512 KB DMAs from the memset tile.
            for c in range(NCH):
                for b in range(NB):
                    dma_engines[b].dma_start(
                        out=out_d[b * 128:(b + 1) * 128,
                                  c * CB:(c + 1) * CB, :],
                        in_=zt[:, :].bitcast(u8))
            # Patch the 256 ones: per batch, partition p writes its
            # 500-byte block at block index idx[p] (= global row * 64 +
            # label // 500).  One index per partition: the multi-index-
            # per-partition form passes CoreSim but writes nothing on HW.
            for b in range(NB):
                nc.gpsimd.indirect_dma_start(
                    out=out_d[:, :, :],
                    out_offset=bass.IndirectOffsetOnAxis(
                        ap=idx[b][:, :1], axis=1),
                    in_=patch[b][:, :],
                    in_offset=None)
    nc.finalize()
    return nc


def kernel(**inputs):
    from concourse.bass_utils import run_bass_kernel_spmd

    if "nc" not in _cache:
        _cache["nc"] = _build_nc()
    nc = _cache["nc"]

    lab = np.asarray(inputs["labels"]).reshape(-1).astype(np.int64)
    in_maps = []
    for i in range(N_CORES):
        shard = lab[i * RPC:(i + 1) * RPC].reshape(NB, 128)  # [NB, 128]
        im = {}
        for b in range(NB):
            lb = shard[b]
            patch = np.zeros((128, BLK), dtype=np.uint8)
            patch[np.arange(128), lb % BLK] = 1
            gidx = (b * 128 + np.arange(128)) * NBLK + lb // BLK
            im[f"patch{b}"] = patch
            im[f"idx{b}"] = gidx.astype(np.int32).reshape(128, 1)
        in_maps.append(im)

    trace = bool(int(os.environ.get("BASS_KERNEL_TRACE", "0")))
    res = run_bass_kernel_spmd(nc, in_maps, list(range(N_CORES)), trace=trace)
    _cache["last_res"] = res

    outs = [res.results[i]["out"].reshape(RPC, V) for i in range(N_CORES)]
    return np.concatenate(outs, axis=0).reshape(B, T, V).astype(np.float32)
